# revision 64
# baseline (speedup 1.0000x reference)
"""MetapathAttentionLayer Trainium2 kernel (v3).

Math (per node n):
    scores[n, m] = sum_d x[m, n, d] * W[d, m]
    att = softmax(relu(scores), axis=m)      (8 metapaths)
    out[n, :] = elu(sum_m att[n, m] * x[m, n, :])

Strategy: shard nodes across 8 cores (data parallel), 12544 padded nodes
per core. SBUF layout: partition p = (m, r), m = metapath (8), r =
node-residue (16); node n = r*784 + c for chunk-column c in [0, 784).

Per group of ~56 chunk-columns (software-pipelined; the kernel is
DMA-bound at ~360 GB/s with ~81us of bus time per core):
  - scores: DVE bf16 multiply (2x mode) against the replicated W rows,
    emitted in `mult_split` chunk-quarters; the full d-reduction runs on
    the otherwise-idle PE as 128 accumulating identity matmuls (rhs = one
    d-column slice per pass) into a [128, ct] f32 PSUM tile — an exact
    f32 sum that frees DVE/Pool of ~50us of tree adds each.
  - softmax: relu+exp on ACT give e1 = exp(relu(s)) = max(exp(s),1); one
    PE matmul with a residue-replication matrix sums e1 over m and
    replicates Z to all partitions; 1/Z via DVE fast reciprocal;
    att = e1 * invZ on DVE (bf16).
  - A-matrix A[p, c*16+r(p)] = att[p, c] via gpsimd local_scatter.
  - pooling: one PE matmul per chunk (lhsT = X-chunk [128, 128d],
    rhs = A-slice [128, 16]) -> U[d, 16] in PSUM; the accumulation group
    per 512-f32 bank is held open (start only on the first matmul).
  - elu via PSUM accumulation: elu(u) = u + relu(-u) + exp(-relu(-u)) - 1.
    ACT computes b = relu(-U), c2 = exp(-b); two identity matmuls on PE
    accumulate b and c2 into the live U banks (closing the group); final
    ACT Copy applies bias -1 and writes bf16.
  - store d-major [128=d, (c, i)] straight to DRAM (1.8KB descriptors,
    no transpose); host un-permutes to [n, d] for free.
"""

import numpy as np
import ml_dtypes

import concourse.bass as bass
import concourse.tile as tile
from concourse import bacc, mybir, library_config
import concourse.bass_utils as bass_utils

F32 = mybir.dt.float32
BF16 = mybir.dt.bfloat16
I16 = mybir.dt.int16
ALU = mybir.AluOpType
ACTF = mybir.ActivationFunctionType

NMETA = 8
N = 100000
D = 128
NCORES = 8
NC_RAW = N // NCORES          # 12500 nodes per core
R = 16                        # node-residue groups on partitions
NB = 784                      # chunk-columns per residue: 16*784 = 12544
NC_PAD = R * NB               # padded nodes per core


def kernel_body(tc, out_d, x_d, wb_d, maskf_d, rep_d, eye_d, sidx_d,
                mult_split=4, tree_split=4, dve_slices=0, a_eng="pool",
                y_comb="pe", out_q="act", load0_split=False, front_first=False,
                emit_order="a", tail_pre=0,
                out_delay=6, tail_tight=0, bufs_x=4, bufs_p=2, bufs_tree=2,
                bufs_small=10, bufs_a=2, bufs_bc=2, bufs_y=8, sizes=None):
    nc = tc.nc
    if sizes is None:
        sizes = [56] * 13 + [40, 16]
    assert sum(sizes) == NB
    offs = [0]
    for s_ in sizes:
        offs.append(offs[-1] + s_)
    NGv = len(sizes)
    from contextlib import ExitStack
    with ExitStack() as ctx:
        const = ctx.enter_context(tc.tile_pool(name="const", bufs=1))
        xpool = ctx.enter_context(tc.tile_pool(name="x", bufs=bufs_x))
        ppool = ctx.enter_context(tc.tile_pool(name="p", bufs=bufs_p))
        tpool = ctx.enter_context(tc.tile_pool(name="tree", bufs=bufs_tree))
        spool = ctx.enter_context(tc.tile_pool(name="small", bufs=bufs_small))
        apool = ctx.enter_context(tc.tile_pool(name="amat", bufs=bufs_a))
        bcpool = ctx.enter_context(tc.tile_pool(name="bc", bufs=bufs_bc))
        ypool = ctx.enter_context(tc.tile_pool(name="y", bufs=bufs_y))
        psU = ctx.enter_context(tc.tile_pool(name="psU", bufs=2, space="PSUM"))
        psZ = ctx.enter_context(tc.tile_pool(name="psZ", bufs=2, space="PSUM"))
        psS = ctx.enter_context(tc.tile_pool(name="psS", bufs=2, space="PSUM"))

        cst = {}

        def load_wb():
            # wb gates the first DVE multiply and eye gates the first PE
            # tree matmul -- both must land before the heavy group loads.
            wb_t = const.tile([128, D], BF16, tag="wb")
            nc.sync.dma_start(wb_t[:], wb_d[:])
            eye_t = const.tile([128, 128], BF16, tag="eye")
            nc.sync.dma_start(eye_t[:], eye_d[:])
            cst.update(wb=wb_t, eye=eye_t)

        def load_consts():
            maskf_t = const.tile([128, R], F32, tag="maskf")
            nc.sync.dma_start(maskf_t[:], maskf_d[:])
            rep_t = const.tile([128, 128], BF16, tag="rep")
            nc.sync.dma_start(rep_t[:], rep_d[:])
            sidx_t = const.tile([128, 56], I16, tag="sidx")
            nc.sync.dma_start(sidx_t[:], sidx_d[:])
            cst.update(maskf=maskf_t, rep=rep_t, sidx=sidx_t)
            if a_eng != "dve":
                nc.gpsimd.load_library(library_config.local_scatter)

        dma_eng = {"act": nc.scalar, "sp": nc.sync, "pool": nc.gpsimd,
                   "dve": nc.vector}[out_q]
        st = [dict() for _ in range(NGv)]
        pending = []

        def load(g):
            ct = sizes[g]
            X = xpool.tile([128, ct * D], BF16, tag="X")
            nc.sync.dma_start(X[:], x_d[:, offs[g] * D:(offs[g] + ct) * D])
            st[g]["Xv"] = X[:].rearrange("p (c d) -> p c d", c=ct)

        def mult_scores(g):
            # scores: DVE bf16 multiply, then d-reduction on PE (accumulate
            # single-column slices into a [128, ct] f32 PSUM tile via
            # identity matmuls). Split into `mult_split` chunk ranges so PE
            # starts while DVE is still multiplying. `dve_slices` d-pairs
            # are pre-added on DVE to offload PE.
            ct = sizes[g]
            Xv = st[g]["Xv"]
            P = ppool.tile([128, ct * D], BF16, tag="P")
            Pv = P[:].rearrange("p (c d) -> p c d", c=ct)
            ds = dve_slices
            if ds:
                PH = tpool.tile([128, ct * ds], BF16, tag="PH")
                PHv = PH[:].rearrange("p (c d) -> p c d", c=ct)
            S = psS.tile([128, ct], F32, tag="S")
            npass = 128 - ds
            nh = max(1, min(mult_split, ct))
            step = (ct + nh - 1) // nh
            nt = max(1, min(tree_split, ct))
            tstep = (ct + nt - 1) // nt
            state = {"t0": 0, "mdone": 0}

            def quarter(c0):
                c1 = min(ct, c0 + step)
                nc.vector.tensor_tensor(
                    out=Pv[:, c0:c1, :], in0=Xv[:, c0:c1, :],
                    in1=cst["wb"][:].unsqueeze(1).broadcast_to(
                        [128, c1 - c0, D]),
                    op=ALU.mult)
                if ds:
                    nc.vector.tensor_tensor(
                        out=PHv[:, c0:c1, :], in0=Pv[:, c0:c1, 0:ds],
                        in1=Pv[:, c0:c1, ds:2 * ds], op=ALU.add)
                state["mdone"] = c1
                while state["t0"] < state["mdone"] and (
                        state["mdone"] - state["t0"] >= tstep
                        or state["mdone"] == ct):
                    t0 = state["t0"]
                    t1 = min(state["mdone"], t0 + tstep)
                    for k in range(npass):
                        rhs = (PHv[:, t0:t1, k:k + 1] if k < ds
                               else Pv[:, t0:t1, ds + k:ds + k + 1])
                        nc.tensor.matmul(out=S[:, t0:t1], lhsT=cst["eye"][:],
                                         rhs=rhs,
                                         start=(k == 0),
                                         stop=(k == npass - 1))
                    state["t0"] = t1

            st[g]["scores"] = S
            return [lambda c0=c0: quarter(c0) for c0 in range(0, ct, step)]

        def softmax_head(g):
            ct = sizes[g]
            rs = spool.tile([128, ct], F32, tag="rs")
            nc.scalar.activation(rs[:], st[g].pop("scores")[:], ACTF.Relu)
            e1 = spool.tile([128, ct], BF16, tag="e1")
            nc.scalar.activation(e1[:], rs[:], ACTF.Exp)
            Z = psZ.tile([128, ct], F32, tag="Z")
            nc.tensor.matmul(out=Z[:], lhsT=cst["rep"][:], rhs=e1[:],
                             start=True, stop=True)
            st[g]["e1"] = e1
            st[g]["Z"] = Z

        def att_abuild(g):
            ct = sizes[g]
            inv = spool.tile([128, ct], F32, tag="inv")
            nc.vector.reciprocal_approx_fast(out=inv[:], in_=st[g].pop("Z")[:])
            att = spool.tile([128, ct], BF16, tag="att")
            nc.vector.tensor_tensor(out=att[:], in0=st[g].pop("e1")[:],
                                    in1=inv[:], op=ALU.mult)
            A = apool.tile([128, R * ct], BF16, tag="A")
            eng = a_eng if a_eng != "alt" else ("dve" if g % 2 == 0 else "pool")
            if eng == "dve":
                # A in (i, c) layout: A[p, i*ct + c] = att[p, c]*maskf[p, i]
                for i in range(R):
                    nc.vector.tensor_scalar(
                        A[:, i * ct:(i + 1) * ct], att[:],
                        cst["maskf"][:, i:i + 1], None, ALU.mult)
                st[g]["Av"] = A[:].rearrange("p (i c) -> p c i", i=R)
            else:
                # A in (c, i) layout via gpsimd scatter: A[p, c*R + r(p)]
                nc.gpsimd.local_scatter(A[:], att[:], cst["sidx"][:, 0:ct],
                                        channels=128, num_elems=ct * R,
                                        num_idxs=ct)
                st[g]["Av"] = A[:].rearrange("p (c i) -> p c i", i=R)

        def pool_mm(g):
            ct = sizes[g]
            Xv = st[g].pop("Xv")
            Av = st[g].pop("Av")
            U = psU.tile([128, ct * R], F32, tag="U")
            if y_comb == "pe":
                # Accumulation group stays open for the ELU accumulates:
                # only the first matmul per 512-f32 PSUM bank sets start.
                for c in range(ct):
                    nc.tensor.matmul(
                        out=U[:, c * R:(c + 1) * R],
                        lhsT=Xv[:, c, :],
                        rhs=Av[:, c, :],
                        start=(c * R) % 512 == 0, stop=False)
            else:
                for c in range(ct):
                    nc.tensor.matmul(
                        out=U[:, c * R:(c + 1) * R],
                        lhsT=Xv[:, c, :],
                        rhs=Av[:, c, :],
                        start=True, stop=True)
            st[g]["U"] = U

        def elu(g):
            # elu(U) = relu(U) + exp(-relu(-U)) - 1.
            ct = sizes[g]
            U = st[g].pop("U")
            b = bcpool.tile([128, ct * R], BF16, tag="b")
            nc.scalar.activation(b[:], U[:], ACTF.Relu, scale=-1.0)
            c2 = bcpool.tile([128, ct * R], BF16, tag="c2")
            nc.scalar.activation(c2[:], b[:], ACTF.Exp, scale=-1.0)
            y = ypool.tile([128, ct * R], BF16, tag="y")
            if y_comb in ("pe", "hyb"):
                # correction terms are accumulated into the U PSUM banks by
                # identity matmuls (bank-aligned pieces close the group),
                # then the final ACT Copy applies -1: y = U + b + c2 - 1.
                # "hyb": pre-add b+c2 on DVE (2x) to halve the PE stream.
                if y_comb == "hyb":
                    d_ = bcpool.tile([128, ct * R], BF16, tag="d_")
                    nc.vector.tensor_tensor(out=d_[:], in0=b[:], in1=c2[:],
                                            op=ALU.add)
                for j in range(0, ct * R, 512):
                    e_ = min(ct * R, j + 512)
                    if y_comb == "hyb":
                        nc.tensor.matmul(out=U[:, j:e_], lhsT=cst["eye"][:],
                                         rhs=d_[:, j:e_], start=False,
                                         stop=True)
                    else:
                        nc.tensor.matmul(out=U[:, j:e_], lhsT=cst["eye"][:],
                                         rhs=c2[:, j:e_], start=False,
                                         stop=False)
                        nc.tensor.matmul(out=U[:, j:e_], lhsT=cst["eye"][:],
                                         rhs=b[:, j:e_], start=False,
                                         stop=True)
                nc.scalar.activation(y[:], U[:], ACTF.Copy, bias=-1.0)
            else:
                a = bcpool.tile([128, ct * R], BF16, tag="a")
                nc.scalar.activation(a[:], U[:], ACTF.Relu)
                if y_comb == "dve1":
                    nc.vector.scalar_tensor_tensor(
                        out=y[:], in0=a[:], scalar=-1.0, in1=c2[:],
                        op0=ALU.add, op1=ALU.add)
                else:
                    s_ = bcpool.tile([128, ct * R], BF16, tag="s_")
                    nc.vector.tensor_tensor(out=s_[:], in0=a[:], in1=c2[:],
                                            op=ALU.add)
                    nc.vector.tensor_scalar(y[:], s_[:], -1.0, None, ALU.add)
            pending.append((out_d[:, offs[g] * R:(offs[g] + ct) * R], y[:]))

        def ok(g):
            return 0 <= g < NGv

        if load0_split:
            # Issue group 0's load as mult_split piece-DMAs with wb after
            # the first piece, so mult(0) can start ~4us earlier.
            ct = sizes[0]
            X = xpool.tile([128, ct * D], BF16, tag="X")
            nh = max(1, min(mult_split, ct))
            step = (ct + nh - 1) // nh
            first = True
            for c0 in range(0, ct, step):
                c1 = min(ct, c0 + step)
                nc.sync.dma_start(X[:, c0 * D:c1 * D],
                                  x_d[:, (offs[0] + c0) * D:
                                      (offs[0] + c1) * D])
                if first:
                    load_wb()
                    first = False
            st[0]["Xv"] = X[:].rearrange("p (c d) -> p c d", c=ct)
        else:
            load(0)
            load_wb()
        # Prefetch the last `tail_pre` (small) groups' X right away: their
        # loads would otherwise sit at the end of the bus stream and push
        # out the start of the drain chain.
        for gp in range(max(1, NGv - tail_pre), NGv):
            load(gp)
        load_consts()
        if ok(1):
            load(1)
        done = set()

        def back_half(g):
            if g in done or not ok(g):
                return
            done.add(g)
            att_abuild(g)
            pool_mm(g)
            elu(g)

        for it in range(NGv + 1):
            if ok(it + 2) and it + 2 < max(1, NGv - tail_pre):
                load(it + 2)
            if pending and it >= out_delay:
                dst, src_ = pending.pop(0)
                dma_eng.dma_start(dst, src_)
            if emit_order == "c" and ok(it):
                qs = mult_scores(it)
                qs[0]()
                if ok(it - 1) and (it - 1) not in done:
                    done.add(it - 1)
                    att_abuild(it - 1)
                    pool_mm(it - 1)
                for q in qs[1:]:
                    q()
                softmax_head(it)
                if ok(it - 1):
                    elu(it - 1)
            elif emit_order == "c":
                if ok(it - 1) and (it - 1) not in done:
                    done.add(it - 1)
                    att_abuild(it - 1)
                    pool_mm(it - 1)
                    elu(it - 1)
            else:
                if front_first and ok(it):
                    for q in mult_scores(it):
                        q()
                    softmax_head(it)
                back_half(it - 1)
                if ok(it):
                    if not front_first:
                        for q in mult_scores(it):
                            q()
                        softmax_head(it)
                    if it >= NGv - tail_tight:
                        back_half(it)
        for dst, src_ in pending:
            dma_eng.dma_start(dst, src_)


def host_inputs(x_np, w_np):
    """Build per-core input maps from full fp32 inputs."""
    in_maps = []
    w_bf = w_np.astype(ml_dtypes.bfloat16)          # [D, NMETA]
    # wb[(m,r), d] = W[d, m]
    wb = np.ascontiguousarray(np.repeat(w_bf.T, R, axis=0))     # [128, D]
    maskf = np.zeros((128, R), dtype=np.float32)
    for p in range(128):
        maskf[p, p % R] = 1.0
    rep = np.zeros((128, 128), dtype=ml_dtypes.bfloat16)
    for p in range(128):
        for m2 in range(NMETA):
            rep[p, m2 * R + (p % R)] = 1.0
    eye = np.eye(128, dtype=ml_dtypes.bfloat16)
    sidx = np.zeros((128, 56), dtype=np.int16)
    for p in range(128):
        for c in range(56):
            sidx[p, c] = c * R + (p % R)

    nc_raw = x_np.shape[1] // NCORES
    for core in range(NCORES):
        xs = x_np[:, core * nc_raw:(core + 1) * nc_raw, :]
        xp = np.zeros((NMETA, NC_PAD, D), dtype=ml_dtypes.bfloat16)
        xp[:, :nc_raw, :] = xs.astype(ml_dtypes.bfloat16)
        # xb[(m, r), (c, d)] = x[m, r*NB + c, d]
        xb = np.ascontiguousarray(
            xp.reshape(NMETA, R, NB * D).reshape(128, NB * D))
        in_maps.append({"x": xb, "wb": wb, "maskf": maskf, "rep": rep,
                        "eye": eye, "sidx": sidx})
    return in_maps


_CACHE = {}


def build(**kw):
    key = tuple(sorted((k, tuple(v) if isinstance(v, list) else v)
                       for k, v in kw.items()))
    if key in _CACHE:
        return _CACHE[key]
    nc = bacc.Bacc("TRN2", target_bir_lowering=False, debug=False,
                   num_devices=NCORES)
    x = nc.dram_tensor("x", [128, NB * D], BF16, kind="ExternalInput").ap()
    wb = nc.dram_tensor("wb", [128, D], BF16, kind="ExternalInput").ap()
    maskf = nc.dram_tensor("maskf", [128, R], F32, kind="ExternalInput").ap()
    rep = nc.dram_tensor("rep", [128, 128], BF16, kind="ExternalInput").ap()
    eye = nc.dram_tensor("eye", [128, 128], BF16, kind="ExternalInput").ap()
    sidx = nc.dram_tensor("sidx", [128, 56], I16, kind="ExternalInput").ap()
    # out is d-major: out[d, c*R + i] = y[node(r=i, c), d]
    out = nc.dram_tensor("out", [128, NB * R], BF16,
                         kind="ExternalOutput").ap()
    with tile.TileContext(nc) as tc:
        kernel_body(tc, out, x, wb, maskf, rep, eye, sidx, **kw)
    nc.compile()
    _CACHE[key] = nc
    return nc


def unpermute(o_core):
    # o_core [128=d, NB*R] with col j = c*R + i  ->  [NC_PAD, D], n = i*NB + c
    return np.ascontiguousarray(
        np.asarray(o_core).reshape(D, NB, R).transpose(2, 1, 0)
        .reshape(NC_PAD, D))


def run(input, W, trace=False, _build_kw=None, **trace_kwargs):
    x_np = np.asarray(input, dtype=np.float32)
    w_np = np.asarray(W, dtype=np.float32)
    nc = build(**(_build_kw or {}))
    in_maps = host_inputs(x_np, w_np)
    res = bass_utils.run_bass_kernel_spmd(
        nc, in_maps, core_ids=list(range(NCORES)), trace=trace, **trace_kwargs)
    nc_raw = x_np.shape[1] // NCORES
    full = np.concatenate(
        [unpermute(res.results[c]["out"])[:nc_raw] for c in range(NCORES)],
        axis=0).astype(np.float32)
    return full, res


def kernel(input, W):
    out, _ = run(input, W, trace=False)
    return out


# revision 70
# speedup vs baseline: 1.0004x; 1.0004x over previous
"""MetapathAttentionLayer Trainium2 kernel (v3).

Math (per node n):
    scores[n, m] = sum_d x[m, n, d] * W[d, m]
    att = softmax(relu(scores), axis=m)      (8 metapaths)
    out[n, :] = elu(sum_m att[n, m] * x[m, n, :])

Strategy: shard nodes across 8 cores (data parallel), 12544 padded nodes
per core. SBUF layout: partition p = (m, r), m = metapath (8), r =
node-residue (16); node n = r*784 + c for chunk-column c in [0, 784).

Per group of ~56 chunk-columns (software-pipelined; the kernel is
DMA-bound at ~360 GB/s with ~81us of bus time per core):
  - scores: DVE bf16 multiply (2x mode) against the replicated W rows,
    emitted in `mult_split` chunk-quarters; the full d-reduction runs on
    the otherwise-idle PE as 128 accumulating identity matmuls (rhs = one
    d-column slice per pass) into a [128, ct] f32 PSUM tile — an exact
    f32 sum that frees DVE/Pool of ~50us of tree adds each.
  - softmax: relu+exp on ACT give e1 = exp(relu(s)) = max(exp(s),1); one
    PE matmul with a residue-replication matrix sums e1 over m and
    replicates Z to all partitions; 1/Z via DVE fast reciprocal;
    att = e1 * invZ on DVE (bf16).
  - A-matrix A[p, c*16+r(p)] = att[p, c] via gpsimd local_scatter.
  - pooling: one PE matmul per chunk (lhsT = X-chunk [128, 128d],
    rhs = A-slice [128, 16]) -> U[d, 16] in PSUM; the accumulation group
    per 512-f32 bank is held open (start only on the first matmul).
  - elu via PSUM accumulation: elu(u) = u + relu(-u) + exp(-relu(-u)) - 1.
    ACT computes b = relu(-U), c2 = exp(-b); two identity matmuls on PE
    accumulate b and c2 into the live U banks (closing the group); final
    ACT Copy applies bias -1 and writes bf16.
  - store d-major [128=d, (c, i)] straight to DRAM (1.8KB descriptors,
    no transpose); host un-permutes to [n, d] for free.
"""

import numpy as np
import ml_dtypes

import concourse.bass as bass
import concourse.tile as tile
from concourse import bacc, mybir, library_config
import concourse.bass_utils as bass_utils

F32 = mybir.dt.float32
BF16 = mybir.dt.bfloat16
I16 = mybir.dt.int16
ALU = mybir.AluOpType
ACTF = mybir.ActivationFunctionType

NMETA = 8
N = 100000
D = 128
NCORES = 8
NC_RAW = N // NCORES          # 12500 nodes per core
R = 16                        # node-residue groups on partitions
NB = 784                      # chunk-columns per residue: 16*784 = 12544
NC_PAD = R * NB               # padded nodes per core


def kernel_body(tc, out_d, x_d, wb_d, maskf_d, rep_d, eye_d, sidx_d,
                mult_split=4, tree_split=4, dve_slices=0, a_eng="pool",
                y_comb="pe", out_q="act", load0_split=False, front_first=False,
                emit_order="a", tail_pre=0, ff_tail=2,
                out_delay=6, tail_tight=2, bufs_x=4, bufs_p=2, bufs_tree=2,
                bufs_small=10, bufs_a=2, bufs_bc=2, bufs_y=8, sizes=None):
    nc = tc.nc
    if sizes is None:
        sizes = [56] * 13 + [40, 16]
    assert sum(sizes) == NB
    offs = [0]
    for s_ in sizes:
        offs.append(offs[-1] + s_)
    NGv = len(sizes)
    from contextlib import ExitStack
    with ExitStack() as ctx:
        const = ctx.enter_context(tc.tile_pool(name="const", bufs=1))
        xpool = ctx.enter_context(tc.tile_pool(name="x", bufs=bufs_x))
        ppool = ctx.enter_context(tc.tile_pool(name="p", bufs=bufs_p))
        tpool = ctx.enter_context(tc.tile_pool(name="tree", bufs=bufs_tree))
        spool = ctx.enter_context(tc.tile_pool(name="small", bufs=bufs_small))
        apool = ctx.enter_context(tc.tile_pool(name="amat", bufs=bufs_a))
        bcpool = ctx.enter_context(tc.tile_pool(name="bc", bufs=bufs_bc))
        ypool = ctx.enter_context(tc.tile_pool(name="y", bufs=bufs_y))
        psU = ctx.enter_context(tc.tile_pool(name="psU", bufs=2, space="PSUM"))
        psZ = ctx.enter_context(tc.tile_pool(name="psZ", bufs=2, space="PSUM"))
        psS = ctx.enter_context(tc.tile_pool(name="psS", bufs=2, space="PSUM"))

        cst = {}

        def load_wb():
            # wb gates the first DVE multiply and eye gates the first PE
            # tree matmul -- both must land before the heavy group loads.
            wb_t = const.tile([128, D], BF16, tag="wb")
            nc.sync.dma_start(wb_t[:], wb_d[:])
            eye_t = const.tile([128, 128], BF16, tag="eye")
            nc.sync.dma_start(eye_t[:], eye_d[:])
            cst.update(wb=wb_t, eye=eye_t)

        def load_consts():
            maskf_t = const.tile([128, R], F32, tag="maskf")
            nc.sync.dma_start(maskf_t[:], maskf_d[:])
            rep_t = const.tile([128, 128], BF16, tag="rep")
            nc.sync.dma_start(rep_t[:], rep_d[:])
            sidx_t = const.tile([128, 56], I16, tag="sidx")
            nc.sync.dma_start(sidx_t[:], sidx_d[:])
            cst.update(maskf=maskf_t, rep=rep_t, sidx=sidx_t)
            if a_eng != "dve":
                nc.gpsimd.load_library(library_config.local_scatter)

        dma_eng = {"act": nc.scalar, "sp": nc.sync, "pool": nc.gpsimd,
                   "dve": nc.vector}[out_q]
        st = [dict() for _ in range(NGv)]
        pending = []

        def load(g):
            ct = sizes[g]
            X = xpool.tile([128, ct * D], BF16, tag="X")
            nc.sync.dma_start(X[:], x_d[:, offs[g] * D:(offs[g] + ct) * D])
            st[g]["Xv"] = X[:].rearrange("p (c d) -> p c d", c=ct)

        def mult_scores(g):
            # scores: DVE bf16 multiply, then d-reduction on PE (accumulate
            # single-column slices into a [128, ct] f32 PSUM tile via
            # identity matmuls). Split into `mult_split` chunk ranges so PE
            # starts while DVE is still multiplying. `dve_slices` d-pairs
            # are pre-added on DVE to offload PE.
            ct = sizes[g]
            Xv = st[g]["Xv"]
            P = ppool.tile([128, ct * D], BF16, tag="P")
            Pv = P[:].rearrange("p (c d) -> p c d", c=ct)
            ds = dve_slices
            if ds:
                PH = tpool.tile([128, ct * ds], BF16, tag="PH")
                PHv = PH[:].rearrange("p (c d) -> p c d", c=ct)
            S = psS.tile([128, ct], F32, tag="S")
            npass = 128 - ds
            # keep pieces >=14 cols: tiny tail groups would otherwise pay
            # 4x the matmul instruction overhead on the critical drain chain
            nh = max(1, min(mult_split, ct // 14, ct))
            step = (ct + nh - 1) // nh
            nt = max(1, min(tree_split, ct))
            tstep = (ct + nt - 1) // nt
            state = {"t0": 0, "mdone": 0}

            def quarter(c0):
                c1 = min(ct, c0 + step)
                nc.vector.tensor_tensor(
                    out=Pv[:, c0:c1, :], in0=Xv[:, c0:c1, :],
                    in1=cst["wb"][:].unsqueeze(1).broadcast_to(
                        [128, c1 - c0, D]),
                    op=ALU.mult)
                if ds:
                    nc.vector.tensor_tensor(
                        out=PHv[:, c0:c1, :], in0=Pv[:, c0:c1, 0:ds],
                        in1=Pv[:, c0:c1, ds:2 * ds], op=ALU.add)
                state["mdone"] = c1
                while state["t0"] < state["mdone"] and (
                        state["mdone"] - state["t0"] >= tstep
                        or state["mdone"] == ct):
                    t0 = state["t0"]
                    t1 = min(state["mdone"], t0 + tstep)
                    for k in range(npass):
                        rhs = (PHv[:, t0:t1, k:k + 1] if k < ds
                               else Pv[:, t0:t1, ds + k:ds + k + 1])
                        nc.tensor.matmul(out=S[:, t0:t1], lhsT=cst["eye"][:],
                                         rhs=rhs,
                                         start=(k == 0),
                                         stop=(k == npass - 1))
                    state["t0"] = t1

            st[g]["scores"] = S
            return [lambda c0=c0: quarter(c0) for c0 in range(0, ct, step)]

        def softmax_head(g):
            ct = sizes[g]
            rs = spool.tile([128, ct], F32, tag="rs")
            nc.scalar.activation(rs[:], st[g].pop("scores")[:], ACTF.Relu)
            e1 = spool.tile([128, ct], BF16, tag="e1")
            nc.scalar.activation(e1[:], rs[:], ACTF.Exp)
            Z = psZ.tile([128, ct], F32, tag="Z")
            nc.tensor.matmul(out=Z[:], lhsT=cst["rep"][:], rhs=e1[:],
                             start=True, stop=True)
            st[g]["e1"] = e1
            st[g]["Z"] = Z

        def att_abuild(g):
            ct = sizes[g]
            inv = spool.tile([128, ct], F32, tag="inv")
            nc.vector.reciprocal_approx_fast(out=inv[:], in_=st[g].pop("Z")[:])
            att = spool.tile([128, ct], BF16, tag="att")
            nc.vector.tensor_tensor(out=att[:], in0=st[g].pop("e1")[:],
                                    in1=inv[:], op=ALU.mult)
            A = apool.tile([128, R * ct], BF16, tag="A")
            if a_eng == "alt":
                eng = "dve" if g % 2 == 0 else "pool"
            elif a_eng == "pool_t":
                # tail groups skip the Pool round-trip (shorter drain chain)
                eng = "dve" if g >= NGv - 2 else "pool"
            else:
                eng = a_eng
            if eng == "dve":
                # A in (i, c) layout: A[p, i*ct + c] = att[p, c]*maskf[p, i]
                for i in range(R):
                    nc.vector.tensor_scalar(
                        A[:, i * ct:(i + 1) * ct], att[:],
                        cst["maskf"][:, i:i + 1], None, ALU.mult)
                st[g]["Av"] = A[:].rearrange("p (i c) -> p c i", i=R)
            else:
                # A in (c, i) layout via gpsimd scatter: A[p, c*R + r(p)]
                nc.gpsimd.local_scatter(A[:], att[:], cst["sidx"][:, 0:ct],
                                        channels=128, num_elems=ct * R,
                                        num_idxs=ct)
                st[g]["Av"] = A[:].rearrange("p (c i) -> p c i", i=R)

        def pool_mm(g):
            ct = sizes[g]
            Xv = st[g].pop("Xv")
            Av = st[g].pop("Av")
            U = psU.tile([128, ct * R], F32, tag="U")
            if y_comb == "pe":
                # Accumulation group stays open for the ELU accumulates:
                # only the first matmul per 512-f32 PSUM bank sets start.
                for c in range(ct):
                    nc.tensor.matmul(
                        out=U[:, c * R:(c + 1) * R],
                        lhsT=Xv[:, c, :],
                        rhs=Av[:, c, :],
                        start=(c * R) % 512 == 0, stop=False)
            else:
                for c in range(ct):
                    nc.tensor.matmul(
                        out=U[:, c * R:(c + 1) * R],
                        lhsT=Xv[:, c, :],
                        rhs=Av[:, c, :],
                        start=True, stop=True)
            st[g]["U"] = U

        def elu(g):
            # elu(U) = relu(U) + exp(-relu(-U)) - 1.
            ct = sizes[g]
            U = st[g].pop("U")
            b = bcpool.tile([128, ct * R], BF16, tag="b")
            nc.scalar.activation(b[:], U[:], ACTF.Relu, scale=-1.0)
            c2 = bcpool.tile([128, ct * R], BF16, tag="c2")
            nc.scalar.activation(c2[:], b[:], ACTF.Exp, scale=-1.0)
            y = ypool.tile([128, ct * R], BF16, tag="y")
            if y_comb in ("pe", "hyb"):
                # correction terms are accumulated into the U PSUM banks by
                # identity matmuls (bank-aligned pieces close the group),
                # then the final ACT Copy applies -1: y = U + b + c2 - 1.
                # "hyb": pre-add b+c2 on DVE (2x) to halve the PE stream.
                if y_comb == "hyb":
                    d_ = bcpool.tile([128, ct * R], BF16, tag="d_")
                    nc.vector.tensor_tensor(out=d_[:], in0=b[:], in1=c2[:],
                                            op=ALU.add)
                for j in range(0, ct * R, 512):
                    e_ = min(ct * R, j + 512)
                    if y_comb == "hyb":
                        nc.tensor.matmul(out=U[:, j:e_], lhsT=cst["eye"][:],
                                         rhs=d_[:, j:e_], start=False,
                                         stop=True)
                    else:
                        nc.tensor.matmul(out=U[:, j:e_], lhsT=cst["eye"][:],
                                         rhs=c2[:, j:e_], start=False,
                                         stop=False)
                        nc.tensor.matmul(out=U[:, j:e_], lhsT=cst["eye"][:],
                                         rhs=b[:, j:e_], start=False,
                                         stop=True)
                nc.scalar.activation(y[:], U[:], ACTF.Copy, bias=-1.0)
            else:
                a = bcpool.tile([128, ct * R], BF16, tag="a")
                nc.scalar.activation(a[:], U[:], ACTF.Relu)
                if y_comb == "dve1":
                    nc.vector.scalar_tensor_tensor(
                        out=y[:], in0=a[:], scalar=-1.0, in1=c2[:],
                        op0=ALU.add, op1=ALU.add)
                else:
                    s_ = bcpool.tile([128, ct * R], BF16, tag="s_")
                    nc.vector.tensor_tensor(out=s_[:], in0=a[:], in1=c2[:],
                                            op=ALU.add)
                    nc.vector.tensor_scalar(y[:], s_[:], -1.0, None, ALU.add)
            pending.append((out_d[:, offs[g] * R:(offs[g] + ct) * R], y[:]))

        def ok(g):
            return 0 <= g < NGv

        if load0_split:
            # Issue group 0's load as mult_split piece-DMAs with wb after
            # the first piece, so mult(0) can start ~4us earlier.
            ct = sizes[0]
            X = xpool.tile([128, ct * D], BF16, tag="X")
            nh = max(1, min(mult_split, ct))
            step = (ct + nh - 1) // nh
            first = True
            for c0 in range(0, ct, step):
                c1 = min(ct, c0 + step)
                nc.sync.dma_start(X[:, c0 * D:c1 * D],
                                  x_d[:, (offs[0] + c0) * D:
                                      (offs[0] + c1) * D])
                if first:
                    load_wb()
                    first = False
            st[0]["Xv"] = X[:].rearrange("p (c d) -> p c d", c=ct)
        else:
            load(0)
            load_wb()
        # Prefetch the last `tail_pre` (small) groups' X right away: their
        # loads would otherwise sit at the end of the bus stream and push
        # out the start of the drain chain.
        for gp in range(max(1, NGv - tail_pre), NGv):
            load(gp)
        load_consts()
        if ok(1):
            load(1)
        done = set()

        def back_half(g):
            if g in done or not ok(g):
                return
            done.add(g)
            att_abuild(g)
            pool_mm(g)
            elu(g)

        for it in range(NGv + 1):
            if ok(it + 2) and it + 2 < max(1, NGv - tail_pre):
                load(it + 2)
            if pending and it >= out_delay:
                dst, src_ = pending.pop(0)
                dma_eng.dma_start(dst, src_)
            if emit_order == "c" and ok(it):
                qs = mult_scores(it)
                qs[0]()
                if ok(it - 1) and (it - 1) not in done:
                    done.add(it - 1)
                    att_abuild(it - 1)
                    pool_mm(it - 1)
                for q in qs[1:]:
                    q()
                softmax_head(it)
                if ok(it - 1):
                    elu(it - 1)
            elif emit_order == "c":
                if ok(it - 1) and (it - 1) not in done:
                    done.add(it - 1)
                    att_abuild(it - 1)
                    pool_mm(it - 1)
                    elu(it - 1)
            else:
                ff = front_first or (ff_tail and it >= NGv - ff_tail)
                if ff and ok(it):
                    for q in mult_scores(it):
                        q()
                    softmax_head(it)
                back_half(it - 1)
                if ok(it):
                    if not ff:
                        for q in mult_scores(it):
                            q()
                        softmax_head(it)
                    if it >= NGv - tail_tight:
                        back_half(it)
        for dst, src_ in pending:
            dma_eng.dma_start(dst, src_)


def host_inputs(x_np, w_np):
    """Build per-core input maps from full fp32 inputs."""
    in_maps = []
    w_bf = w_np.astype(ml_dtypes.bfloat16)          # [D, NMETA]
    # wb[(m,r), d] = W[d, m]
    wb = np.ascontiguousarray(np.repeat(w_bf.T, R, axis=0))     # [128, D]
    maskf = np.zeros((128, R), dtype=np.float32)
    for p in range(128):
        maskf[p, p % R] = 1.0
    rep = np.zeros((128, 128), dtype=ml_dtypes.bfloat16)
    for p in range(128):
        for m2 in range(NMETA):
            rep[p, m2 * R + (p % R)] = 1.0
    eye = np.eye(128, dtype=ml_dtypes.bfloat16)
    sidx = np.zeros((128, 56), dtype=np.int16)
    for p in range(128):
        for c in range(56):
            sidx[p, c] = c * R + (p % R)

    nc_raw = x_np.shape[1] // NCORES
    for core in range(NCORES):
        xs = x_np[:, core * nc_raw:(core + 1) * nc_raw, :]
        xp = np.zeros((NMETA, NC_PAD, D), dtype=ml_dtypes.bfloat16)
        xp[:, :nc_raw, :] = xs.astype(ml_dtypes.bfloat16)
        # xb[(m, r), (c, d)] = x[m, r*NB + c, d]
        xb = np.ascontiguousarray(
            xp.reshape(NMETA, R, NB * D).reshape(128, NB * D))
        in_maps.append({"x": xb, "wb": wb, "maskf": maskf, "rep": rep,
                        "eye": eye, "sidx": sidx})
    return in_maps


_CACHE = {}


def build(**kw):
    key = tuple(sorted((k, tuple(v) if isinstance(v, list) else v)
                       for k, v in kw.items()))
    if key in _CACHE:
        return _CACHE[key]
    nc = bacc.Bacc("TRN2", target_bir_lowering=False, debug=False,
                   num_devices=NCORES)
    x = nc.dram_tensor("x", [128, NB * D], BF16, kind="ExternalInput").ap()
    wb = nc.dram_tensor("wb", [128, D], BF16, kind="ExternalInput").ap()
    maskf = nc.dram_tensor("maskf", [128, R], F32, kind="ExternalInput").ap()
    rep = nc.dram_tensor("rep", [128, 128], BF16, kind="ExternalInput").ap()
    eye = nc.dram_tensor("eye", [128, 128], BF16, kind="ExternalInput").ap()
    sidx = nc.dram_tensor("sidx", [128, 56], I16, kind="ExternalInput").ap()
    # out is d-major: out[d, c*R + i] = y[node(r=i, c), d]
    out = nc.dram_tensor("out", [128, NB * R], BF16,
                         kind="ExternalOutput").ap()
    with tile.TileContext(nc) as tc:
        kernel_body(tc, out, x, wb, maskf, rep, eye, sidx, **kw)
    nc.compile()
    _CACHE[key] = nc
    return nc


def unpermute(o_core):
    # o_core [128=d, NB*R] with col j = c*R + i  ->  [NC_PAD, D], n = i*NB + c
    return np.ascontiguousarray(
        np.asarray(o_core).reshape(D, NB, R).transpose(2, 1, 0)
        .reshape(NC_PAD, D))


def run(input, W, trace=False, _build_kw=None, **trace_kwargs):
    x_np = np.asarray(input, dtype=np.float32)
    w_np = np.asarray(W, dtype=np.float32)
    nc = build(**(_build_kw or {}))
    in_maps = host_inputs(x_np, w_np)
    res = bass_utils.run_bass_kernel_spmd(
        nc, in_maps, core_ids=list(range(NCORES)), trace=trace, **trace_kwargs)
    nc_raw = x_np.shape[1] // NCORES
    full = np.concatenate(
        [unpermute(res.results[c]["out"])[:nc_raw] for c in range(NCORES)],
        axis=0).astype(np.float32)
    return full, res


def kernel(input, W):
    out, _ = run(input, W, trace=False)
    return out
